# revision 27
# baseline (speedup 1.0000x reference)
"""Trainium2 Bass kernel for nn_Attn_88691074662550.

Reference computation (jax):
    energy = enc @ W.T + b          # [S, H]
    scores = energy @ hidden        # [S]
    attn   = softmax(scores)        # [1, S]

Algebraic collapse:
    scores = enc @ (W.T @ hidden) + (b . hidden)
and softmax is shift-invariant, so the constant (b . hidden) drops out:
    attn = softmax(enc @ u),  u = W.T @ hidden

Memory-bound: one streaming pass over the 256 MB encoder_outputs.
Sharding: encoder_outputs split along seq_len across 8 cores (32768 rows /
core); W and hidden replicated; softmax segmented per-core with the 8
per-core (max, sum) pairs combined via a tiny AllGather on device.

Per-core compute structure (this is the part rewritten vs the first
working version, which was per-row-instruction-bound at ~2x the DMA
floor):
  - DVE multiplies whole half-chunks at once: prods[P, 16, 256] =
    xt * u_bc (one big tensor_tensor per half-chunk, ~0.74 ns/elem).
  - Rows are then reduced 256->1 two ways, split across engines so no
    engine exceeds the ~97us DMA streaming floor:
      * nact rows/chunk on ACT: per-row Copy activation with accum_out
        (830 ns/row, fixed overhead dominated).
      * the rest on DVE: a strided binary add-tree 256->128->64->32 (one
        instruction per level covering all rows at once) + one
        tensor_reduce 32->1.
  - softmax uses a single per-core max (m1) instead of per-partition
    shifts: exp underflow for entries far below the core max is harmless
    since those entries are ~0 relative to the peak anyway.

Only standard BIR instructions are used (the walrus build here rejects
bass_isa extended ISA ops), and a post-pass spills any instruction's
second-and-later sync waits into standalone EventSemaphore instructions
(the instruction structs only fit one embedded wait).
"""

import numpy as np

S = 262144
H = 256
NCORES = 8
SHARD = S // NCORES          # 32768 rows per core
P = 128                      # SBUF partitions
RPP = SHARD // P             # 256 rows per partition

_CACHE = {}


def _build(shard=SHARD, nchunk=8, nact=8, npool=0):
    """Build the Bass program (same program runs SPMD on all 8 cores).

    nchunk: number of streaming chunks (DMA granularity = 32MB/nchunk).
    nact:   rows per chunk reduced on ACT (per-row Copy+accum); their
            elementwise multiplies run batched on Pool (npool of them)
            or DVE (the rest).  The remaining nrc-nact rows are fused
            mult+sum-accumulate scalar_tensor_tensor rows on DVE.
    npool:  how many of the nact ACT rows are Pool-multiplied.
    """
    import concourse.bass as bass
    import concourse.tile as tile
    from concourse import mybir

    rpp = shard // P              # rows per partition
    nrc = rpp // nchunk           # rows per partition per chunk
    assert rpp % nchunk == 0
    half = nrc // 2
    ndve = nrc - nact             # DVE fused rows per chunk
    assert ndve >= 0 and 0 <= npool <= nact
    f32 = mybir.dt.float32
    f16 = mybir.dt.float16
    Alu = mybir.AluOpType
    Act = mybir.ActivationFunctionType
    Axis = mybir.AxisListType

    nc = bass.Bass(num_devices=NCORES)

    enc = nc.declare_dram_parameter("enc", [shard, H], f32, isOutput=False)
    w = nc.declare_dram_parameter("w", [H, H], f32, isOutput=False)
    hid = nc.declare_dram_parameter("hid", [1, H], f32, isOutput=False)
    attn = nc.declare_dram_parameter("attn", [1, shard], f32, isOutput=True)

    def rep_ap(ap, n):
        """[P, F] AP -> [P, n, F] with the middle dim 0-strided (repeat)."""
        return bass.AP(
            tensor=ap.tensor, offset=ap.offset, ap=[ap.ap[0], [0, n]] + ap.ap[1:]
        )

    with tile.TileContext(nc) as tc:
        with (
            tc.tile_pool(name="singles", bufs=1) as singles,
            tc.tile_pool(name="chunks", bufs=5) as chunks,
            tc.tile_pool(name="prods", bufs=3) as prodp,
            tc.tile_pool(name="tree", bufs=2) as treep,
            tc.tile_pool(name="stats", bufs=1) as stats,
            tc.tile_pool(name="psum", bufs=1, space="PSUM") as psum,
            tc.tile_pool(name="dram", bufs=1, space="DRAM") as dram,
        ):
            # Streaming blocks: 7 full chunks + the last chunk as two
            # halves (shorter post-stream tail).  Issue the first block's
            # DMAs before anything else so the HBM stream starts
            # immediately; the u computation below overlaps it.
            rpp_blocks = [(c * nrc, nrc) for c in range(nchunk - 1)]
            rpp_blocks += [
                ((nchunk - 1) * nrc, half),
                ((nchunk - 1) * nrc + half, nrc - half),
            ]
            enc_r = enc[:].rearrange("(p r) h -> p r h", p=P)
            xts = {}
            DQ = 8

            def emit_block_dma(bi):
                base, n = rpp_blocks[bi]
                xt = chunks.tile([P, n, H], f16, tag="xt")
                xts[bi] = xt
                for lo in range(0, n, DQ):
                    hi = min(n, lo + DQ)
                    nc.gpsimd.dma_start(
                        out=xt[:, lo:hi, :],
                        in_=enc_r[:, base + lo : base + hi, :],
                    )

            emit_block_dma(0)

            # ---- u = W.T @ hidden on PE; broadcast via ones-matmul ----
            # W rows k = kk*128 + p live at partition p, free slot kk.
            w_sb = singles.tile([P, 2, H], f32)
            nc.gpsimd.dma_start(
                out=w_sb, in_=w[:].rearrange("(kk p) h -> p kk h", kk=2)
            )
            hid_sb = singles.tile([P, 2], f32)
            nc.gpsimd.dma_start(
                out=hid_sb, in_=hid[0, :].rearrange("(kk p) -> p kk", kk=2)
            )
            ones_r = singles.tile([1, P], f32)
            nc.vector.memset(ones_r, 1.0)
            psum_u = psum.tile([1, H], f32)
            for kk in range(2):
                nc.tensor.matmul(
                    out=psum_u,
                    lhsT=hid_sb[:, kk : kk + 1],
                    rhs=w_sb[:, kk, :],
                    start=(kk == 0),
                    stop=(kk == 1),
                )
            u_row = singles.tile([1, H], f32)
            nc.vector.tensor_copy(u_row, psum_u)
            psum_bc = psum.tile([P, H], f32)
            nc.tensor.matmul(
                out=psum_bc, lhsT=ones_r, rhs=u_row, start=True, stop=True
            )
            u_bc = singles.tile([P, H], f32)
            nc.vector.tensor_copy(u_bc, psum_bc)
            # fp16 copy of u for the 2x-rate fp16 dot-product path
            u16 = singles.tile([P, H], f16)
            nc.vector.tensor_copy(u16, psum_bc)

            # Warm the exp table set early so the ~2.7us ACT_TABLE_LOAD
            # overlaps streaming instead of sitting in the softmax tail.
            warm = stats.tile([P, 1], f32)
            nc.scalar.activation(
                out=warm, in_=u_bc[:, 0:1], func=Act.Exp, bias=0.0, scale=0.0
            )

            # ---- stream encoder shard ----
            # Per-row fused dot products (mult + sum-accumulate in one
            # pass, 2KB of SBUF reads per row and no intermediate tile)
            # spread across three engines:
            #   rows [0, ndve)            DVE scalar_tensor_tensor
            #   rows [ndve, ndve+nact)    DVE batched multiply -> prods,
            #                             then ACT Copy+accum per row
            #   rows [ndve+nact, nrc)     GPSIMD scalar_tensor_tensor
            # The last chunk is processed as two half-chunks so the
            # post-stream tail only serializes half a chunk of compute.
            scores = singles.tile([P, rpp], f32)
            # ACT's throwaway output stream goes to PSUM: the ScalarE write
            # port is faster toward PSUM than SBUF.
            dump_a = psum.tile([P, H], f32)

            def emit_block_compute(bi):
                base, n = rpp_blocks[bi]
                na = max(0, min(n, round(nact * n / nrc)))
                nd = n - na
                xt = xts[bi]
                prods = prodp.tile([P, n, H], f16, tag="prods")
                # one multiply per DMA granule so compute starts as soon
                # as each granule lands
                for lo in range(0, n, DQ):
                    hi = min(n, lo + DQ)
                    nc.vector.tensor_mul(
                        prods[:, lo:hi, :],
                        xt[:, lo:hi, :],
                        rep_ap(u16[:], hi - lo),
                    )
                # rows [0, nd): fp16 add-tree 256->128->64->32, then a
                # 32->1 reduce into fp32 scores
                t1 = treep.tile([P, nd, 128], f16, tag="t1")
                nc.vector.tensor_add(
                    t1, prods[:, 0:nd, 0:128], prods[:, 0:nd, 128:256]
                )
                t2 = treep.tile([P, nd, 64], f16, tag="t2")
                nc.vector.tensor_add(t2, t1[:, :, 0:64], t1[:, :, 64:128])
                t3 = treep.tile([P, nd, 32], f16, tag="t3")
                nc.vector.tensor_add(t3, t2[:, :, 0:32], t2[:, :, 32:64])
                nc.vector.tensor_reduce(
                    out=scores[:, base : base + nd],
                    in_=t3,
                    axis=Axis.X,
                    op=Alu.add,
                )
                # rows [nd, n): per-row ACT Copy+accum from fp16 prods
                for j in range(na):
                    col = base + nd + j
                    nc.scalar.activation(
                        out=dump_a,
                        in_=prods[:, nd + j, :],
                        func=Act.Copy,
                        bias=0.0,
                        scale=1.0,
                        accum_out=scores[:, col : col + 1],
                    )

            for bi in range(len(rpp_blocks)):
                if bi > 0:
                    emit_block_dma(bi)
                emit_block_compute(bi)

            # ---- per-core softmax with a single core-wide max shift ----
            m_p = stats.tile([P, 1], f32)
            nc.vector.tensor_reduce(out=m_p, in_=scores, axis=Axis.X, op=Alu.max)
            m1 = stats.tile([1, 1], f32)
            nc.gpsimd.tensor_reduce(out=m1, in_=m_p, axis=Axis.C, op=Alu.max)
            # broadcast m1 to all partitions on PE: ones[1,128].T @ m1[1,1]
            psum_m = psum.tile([P, 1], f32)
            nc.tensor.matmul(out=psum_m, lhsT=ones_r, rhs=m1, start=True, stop=True)
            m_bc = stats.tile([P, 1], f32)
            nc.vector.tensor_copy(m_bc, psum_m)
            neg_mbc = stats.tile([P, 1], f32)
            nc.scalar.mul(out=neg_mbc, in_=psum_m, mul=-1.0)
            exp_sb = singles.tile([P, rpp], f32)
            s_p = stats.tile([P, 1], f32)
            nc.scalar.activation(
                out=exp_sb, in_=scores, func=Act.Exp, bias=neg_mbc, scale=1.0,
                accum_out=s_p,
            )
            # S = sum_p s_p on PE
            ones_c = singles.tile([P, 1], f32)
            nc.vector.memset(ones_c, 1.0)
            psum_s = psum.tile([1, 1], f32)
            nc.tensor.matmul(out=psum_s, lhsT=s_p, rhs=ones_c, start=True, stop=True)

            # ---- AllGather the 8 (max, sum) pairs ----
            pack = stats.tile([1, 2], f32)
            nc.vector.tensor_copy(pack[:, 0:1], m1)
            nc.vector.tensor_copy(pack[:, 1:2], psum_s)
            cc_in = dram.tile([1, 2], f32)
            cc_out = dram.tile([1, 2 * NCORES], f32)
            nc.sync.dma_start(out=cc_in[:], in_=pack)
            nc.gpsimd.collective_compute(
                "AllGather",
                Alu.bypass,
                replica_groups=[list(range(NCORES))],
                ins=[cc_in[:]],
                outs=[cc_out[:]],
            )
            g1 = stats.tile([1, 2 * NCORES], f32)
            nc.sync.dma_start(out=g1, in_=cc_out[:])
            psum_g = psum.tile([P, 2 * NCORES], f32)
            nc.tensor.matmul(out=psum_g, lhsT=ones_r, rhs=g1, start=True, stop=True)
            g = stats.tile([P, NCORES, 2], f32)
            nc.vector.tensor_copy(g, psum_g[:].rearrange("p (c t) -> p c t", c=NCORES))

            # ---- global (max, sum); per-partition scale factor ----
            m_vec = g[:, :, 0]
            s_vec = g[:, :, 1]
            m_glob = stats.tile([P, 1], f32)
            nc.vector.tensor_reduce(out=m_glob, in_=m_vec, axis=Axis.X, op=Alu.max)
            neg_mg = stats.tile([P, 1], f32)
            nc.scalar.mul(out=neg_mg, in_=m_glob, mul=-1.0)
            t8 = stats.tile([P, NCORES], f32)
            nc.scalar.activation(
                out=t8, in_=m_vec, func=Act.Exp, bias=neg_mg, scale=1.0
            )
            z = stats.tile([P, 1], f32)
            dump8 = stats.tile([P, 1], f32)
            nc.vector.scalar_tensor_tensor(
                out=dump8.broadcast_to((P, NCORES)),
                in0=t8,
                scalar=0.0,
                in1=s_vec,
                op0=Alu.bypass,
                op1=Alu.mult,
                accum_out=z,
            )
            # alpha = exp(m1 - m_glob) / z  (same for all partitions)
            e_a = stats.tile([P, 1], f32)
            nc.scalar.activation(
                out=e_a, in_=m_bc, func=Act.Exp, bias=neg_mg, scale=1.0
            )
            rz = stats.tile([P, 1], f32)
            nc.vector.reciprocal(rz, z)
            alpha = stats.tile([P, 1], f32)
            nc.vector.tensor_mul(alpha, e_a, rz)

            # ---- final normalize and store ----
            final = singles.tile([P, rpp], f32)
            nc.vector.tensor_scalar_mul(final, exp_sb, alpha)
            nc.sync.dma_start(
                out=attn[0, :].rearrange("(p r) -> p r", p=P), in_=final
            )

    return nc


def _split_excess_waits(nc, mybir):
    """The walrus codegen here allows only one embedded sync wait on most
    instruction structs (STT, Matmult LW, Drain, ...). Spill extra waits into
    standalone EventSemaphore instructions placed just before, on the same
    engine — semantically identical, since all waits must pass before the
    instruction issues."""
    n = 0
    for fn in nc.m.functions:
        for blk in fn.blocks:
            out = []
            for inst in blk.instructions:
                si = inst.sync_info
                if (
                    si is not None
                    and si.on_wait
                    and len(si.on_wait) > 1
                    and inst.opcode not in ("EventSemaphore", "NoOp")
                ):
                    for wt in si.on_wait[:-1]:
                        n += 1
                        ev = mybir.InstEventSemaphore(
                            name=f"EVSPILL-{n}", ins=[], outs=[]
                        )
                        ev.engine = inst.engine
                        ev.sync_info = mybir.SyncInfo(on_wait=[wt], on_update=[])
                        out.append(ev)
                    si.on_wait = si.on_wait[-1:]
                out.append(inst)
            blk.instructions = out
    return nc


def _get_nc(shard=SHARD, nchunk=8, nact=8, npool=0):
    key = (shard, nchunk, nact, npool)
    if key not in _CACHE:
        _CACHE[key] = _build(shard, nchunk, nact, npool)
    return _CACHE[key]


def run(inputs, trace=False, shard=SHARD, nchunk=8, nact=8, npool=0):
    """Run on hardware. Returns (attn [1, S], BassKernelResults)."""
    from concourse.bass_utils import run_bass_kernel_spmd

    nc = _get_nc(shard, nchunk, nact, npool)
    if not getattr(nc, "_waits_split", False):
        from concourse import mybir

        _split_excess_waits(nc, mybir)
        nc._waits_split = True
    enc_full = np.ascontiguousarray(inputs["encoder_outputs"], dtype=np.float32)
    w_full = np.ascontiguousarray(inputs["W"], dtype=np.float32)
    hid_full = np.ascontiguousarray(
        inputs["hidden"], dtype=np.float32
    ).reshape(1, H)
    n = enc_full.shape[0] // NCORES
    assert n == shard, f"expected shard {shard}, got {n}"
    in_maps = [
        {
            "enc": np.ascontiguousarray(enc_full[i * n : (i + 1) * n]),
            "w": w_full,
            "hid": hid_full,
        }
        for i in range(NCORES)
    ]
    res = run_bass_kernel_spmd(
        nc, in_maps, core_ids=list(range(NCORES)), trace=trace
    )
    out = np.concatenate([r["attn"] for r in res.results], axis=1)
    return out, res


def kernel(**inputs) -> np.ndarray:
    out, _ = run(inputs)
    return out
